# revision 1
# baseline (speedup 1.0000x reference)
"""Trainium2 Bass kernel for ConeProjection.

Math (per batch element b):
    W     = [R[:,0], R[:,1], t - eyes]          (3 rows)
    d_a   = v . W_a          (unnormalized)
    G_ac  = W_a . W_c
    inv2  = 1 / max(||v||^2, eps)
    s     = (d d^T) * inv2 - alpha * G          (6 unique entries)
    out[k] = s . q[k],  q[k] = [x^2, y^2, 1, 2xy, 2x, 2y]  (169 grid pts)

Strategy: pure data-parallel over 8 NeuronCores (batch 131072 -> 16384/core).
Per core, partition p holds batch [p*NI, (p+1)*NI); within-partition index i.
Inputs are cast to fp16 during the load DMA (SWDGE); the elementwise phase
runs fp16 on DVE/ACT/Pool (the 1/||v||^2 path stays fp32 for tail
robustness). The 6 sigma entries land in an AoS tile S[p, 6*i+c] (fp16); PE
transposes S chunks to S^T tiles (components on partitions) which become
fp16 matmul stationary weights against a constant block-diagonal fp16 Q
(3 batch rows per matmul, K=18, N=507). The 3 matmuls of a block write one
3-bank PSUM tile (fp32) which a single strided copy converts to an fp16
staging tile, stored batch-major with one DMA per block. Output returns
fp16 and is upcast to fp32 on the host; end-to-end rel err ~1.2e-3 vs the
fp32 reference (gate 2e-2).
"""

from contextlib import ExitStack

import numpy as np

import concourse.bass as bass
import concourse.bacc as bacc
import concourse.tile as tile
from concourse import mybir
from concourse.bass_utils import run_bass_kernel_spmd

N_CORES = 8
B = 131072
BC = B // N_CORES          # 16384 per core
P = 128                    # partitions
NI = BC // P               # 128 within-partition batch indices
KG = 169                   # grid points
F32 = mybir.dt.float32
F16 = mybir.dt.float16

# i's are processed in groups of 3 (one matmul per group, K=18, N=507);
# 3 groups per PE-transpose block (output partitions 0/32/64 -> matmul
# base-partition constraint).
GROUP = 3
N_FULL_GROUPS = NI // GROUP        # 42
REM_I = NI - GROUP * N_FULL_GROUPS  # 2
BLOCK_GROUPS = 3
N_BLOCKS = N_FULL_GROUPS // BLOCK_GROUPS  # 14
assert N_FULL_GROUPS % BLOCK_GROUPS == 0

# tuning knobs (read at build time)
CHUNKS = (2, 5, 7)         # blocks per elementwise chunk (ramped)
COPY_MOD = 2               # every COPY_MOD-th block out-copy goes to ACT
PACK_ON_ACT = True         # transpose pack-copy engine
ST_ON_ACT = True           # S^T PSUM->SBUF copies on ACT
SQUARES_ON_ACT = True      # self-dot muls (x*x) via ACT Square
RED_ON_POOL = 0            # Pool can't do free-axis reduce (keep 0)
AG_ON_POOL = True          # alpha*G mul on Pool
PSM_BUFS = 2               # PSUM 3-bank matmul-out tiles
STAGE_BUFS = 4             # SBUF output staging buffers
ST_BUFS = 4                # S^T SBUF buffers


def _grid_q():
    ii, jj = np.meshgrid(np.arange(13), np.arange(13), indexing="ij")
    x = ((ii - 6) / 6.0).reshape(-1)
    y = ((jj - 6) / 6.0).reshape(-1)
    q = np.stack([x * x, y * y, np.ones(KG), 2 * x * y, 2 * x, 2 * y], axis=0)
    return q.astype(np.float16)  # [6, 169]


def _q_blockdiag(q, m):
    out = np.zeros((6 * m, KG * m), np.float16)
    for a in range(m):
        out[6 * a : 6 * a + 6, KG * a : KG * a + KG] = q
    return out


def make_q96():
    """[96, 508]: K=18 block-diag Q replicated at partition bases 0/32/64."""
    q18 = _q_blockdiag(_grid_q(), 3)  # [18, 507]
    out = np.zeros((96, 508), np.float16)
    for g in range(3):
        out[32 * g : 32 * g + 18, 0:507] = q18
    return out


def make_q12():
    return _q_blockdiag(_grid_q(), 2)  # [12, 338]


def build_nc(reps: int = 1, loop_n: int = 0):
    nc = bacc.Bacc("TRN2", target_bir_lowering=False, debug=False,
                   num_devices=N_CORES)

    eyes_d = nc.declare_dram_parameter("eyes", [BC, 3], F32, isOutput=False)
    v_d = nc.declare_dram_parameter("v", [BC, 3], F32, isOutput=False)
    r_d = nc.declare_dram_parameter("R", [BC, 3, 3], F32, isOutput=False)
    t_d = nc.declare_dram_parameter("t", [BC, 3], F32, isOutput=False)
    a_d = nc.declare_dram_parameter("alpha", [BC], F32, isOutput=False)
    q96_d = nc.declare_dram_parameter("q96", [96, 508], F16, isOutput=False)
    q12_d = nc.declare_dram_parameter("q12", [12, 338], F16, isOutput=False)
    id_d = nc.declare_dram_parameter("ident", [P, P], F16, isOutput=False)
    out_d = nc.declare_dram_parameter("out", [BC, KG], F16, isOutput=True)

    with tile.TileContext(nc) as tc:
        with ExitStack() as ctx:
            const = ctx.enter_context(tc.tile_pool(name="const", bufs=1))
            q96_sb = const.tile([96, 508], F16)
            nc.sync.dma_start(q96_sb[:], q96_d.ap())
            q12_sb = const.tile([12, 338], F16)
            nc.sync.dma_start(q12_sb[:], q12_d.ap())
            id_sb = const.tile([P, P], F16)
            nc.sync.dma_start(id_sb[:], id_d.ap())

            pools = dict(
                io=ctx.enter_context(tc.tile_pool(name="io", bufs=2)),
                scr=ctx.enter_context(tc.tile_pool(name="scr", bufs=2)),
                tmpp=ctx.enter_context(tc.tile_pool(name="tmp", bufs=2)),
                stp=ctx.enter_context(tc.tile_pool(name="st", bufs=ST_BUFS)),
                stagep=ctx.enter_context(tc.tile_pool(name="stage", bufs=STAGE_BUFS)),
                pkp=ctx.enter_context(tc.tile_pool(name="pk", bufs=3)),
                psq=ctx.enter_context(tc.tile_pool(name="psq", bufs=2, space="PSUM")),
                psm=ctx.enter_context(tc.tile_pool(name="psm", bufs=PSM_BUFS, space="PSUM")),
            )
            if loop_n:
                with tc.For_i(0, loop_n, 1):
                    _emit_one_pass(nc, tc, pools,
                                   eyes_d, v_d, r_d, t_d, a_d, out_d,
                                   q96_sb, q12_sb, id_sb)
            else:
                for _ in range(reps):
                    _emit_one_pass(nc, tc, pools,
                                   eyes_d, v_d, r_d, t_d, a_d, out_d,
                                   q96_sb, q12_sb, id_sb)

    nc.compile()
    return nc


def _emit_one_pass(nc, tc, pools,
                   eyes_d, v_d, r_d, t_d, a_d, out_d,
                   q96_sb, q12_sb, id_sb):
    with ExitStack() as lpctx:
        lpctx.enter_context(
            nc.allow_low_precision(reason="fp16 kernel validated vs fp32 ref"))
        _emit_one_pass_lp(nc, tc, pools, eyes_d, v_d, r_d, t_d, a_d, out_d,
                          q96_sb, q12_sb, id_sb)


def _emit_one_pass_lp(nc, tc, pools,
                      eyes_d, v_d, r_d, t_d, a_d, out_d,
                      q96_sb, q12_sb, id_sb):
    X = mybir.AxisListType.X
    ADD = mybir.AluOpType.add

    io = pools["io"]
    scr = pools["scr"]
    tmpp = pools["tmpp"]
    stp = pools["stp"]
    stagep = pools["stagep"]
    pkp = pools["pkp"]
    psq = pools["psq"]
    psm = pools["psm"]

    # DRAM views (per-partition contiguous)
    eyes_f = eyes_d.ap().rearrange("(p i) c -> p (i c)", p=P)
    v_f = v_d.ap().rearrange("(p i) c -> p (i c)", p=P)
    r_f = r_d.ap().rearrange("(p i) a b -> p (i a b)", p=P)
    t_f = t_d.ap().rearrange("(p i) c -> p (i c)", p=P)
    out_flat = out_d.ap().rearrange("(p i) k -> p (i k)", p=P)  # [P, NI*KG]

    a_sb = io.tile([P, NI], F16)
    nc.gpsimd.dma_start(a_sb[:], a_d.ap().rearrange("(p i) -> p i", p=P))

    # chunks: (i0, i1, blk0, blk1, has_rem)
    assert sum(CHUNKS) == N_BLOCKS
    chunks = []
    b0 = 0
    for nb in CHUNKS:
        b1 = b0 + nb
        last = b1 == N_BLOCKS
        chunks.append((9 * b0, NI if last else 9 * b1, b0, b1, last))
        b0 = b1
    copy_idx = 0
    for ci, (i0, i1, blk0, blk1, has_rem) in enumerate(chunks):
        ni = i1 - i0

        eyes_sb = io.tile([P, 3 * ni], F16, tag=f"eyes{ci}")
        nc.gpsimd.dma_start(eyes_sb[:], eyes_f[:, 3 * i0 : 3 * i1])
        v_sb = io.tile([P, 3 * ni], F16, tag=f"v{ci}")
        nc.gpsimd.dma_start(v_sb[:], v_f[:, 3 * i0 : 3 * i1])
        r_sb = io.tile([P, 9 * ni], F16, tag=f"r{ci}")
        nc.gpsimd.dma_start(r_sb[:], r_f[:, 9 * i0 : 9 * i1])
        t_sb = io.tile([P, 3 * ni], F16, tag=f"t{ci}")
        nc.gpsimd.dma_start(t_sb[:], t_f[:, 3 * i0 : 3 * i1])

        w2_sb = scr.tile([P, 3 * ni], F16, tag=f"w2_{ci}")
        d_aos = scr.tile([P, 3 * ni], F16, tag=f"d_{ci}")
        g_aos = scr.tile([P, 6 * ni], F16, tag=f"g_{ci}")
        ag = scr.tile([P, 6 * ni], F16, tag=f"ag_{ci}")
        s_aos = scr.tile([P, 6 * ni + 16], F16, tag=f"s_{ci}")
        nv2 = scr.tile([P, ni], F32, tag=f"nv2_{ci}")
        inv2 = scr.tile([P, ni], F16, tag=f"inv2_{ci}")

        v3 = v_sb[:].rearrange("p (i c) -> p i c", c=3)
        w23 = w2_sb[:].rearrange("p (i c) -> p i c", c=3)
        r9 = r_sb[:].rearrange("p (i a b) -> p i a b", a=3, b=3)
        d3 = d_aos[:].rearrange("p (i a) -> p i a", a=3)
        g6 = g_aos[:].rearrange("p (i g) -> p i g", g=6)
        ag6 = ag[:].rearrange("p (i g) -> p i g", g=6)
        s6 = s_aos[:, 0 : 6 * ni].rearrange("p (i c) -> p i c", c=6)
        w0 = r9[:, :, :, 0]
        w1 = r9[:, :, :, 1]

        nc.vector.tensor_sub(w2_sb[:], t_sb[:], eyes_sb[:])

        # (a, b, out): vv reduces in fp32 for the norm path
        dots = [
            (v3, v3, nv2[:]),
            (v3, w0, d3[:, :, 0]),
            (v3, w1, d3[:, :, 1]),
            (v3, w23, d3[:, :, 2]),
            (w0, w0, g6[:, :, 0]),
            (w1, w1, g6[:, :, 1]),
            (w23, w23, g6[:, :, 2]),
            (w0, w1, g6[:, :, 3]),
            (w0, w23, g6[:, :, 4]),
            (w1, w23, g6[:, :, 5]),
        ]
        for di, (a_v, b_v, o_v) in enumerate(dots):
            tmp = tmpp.tile([P, 3 * ni], F16, tag=f"dm{ci}")
            t3 = tmp[:].rearrange("p (i c) -> p i c", c=3)
            if SQUARES_ON_ACT and a_v is b_v:
                nc.scalar.square(t3, a_v)
            else:
                nc.vector.tensor_mul(t3, a_v, b_v)
            red_eng = nc.gpsimd if di < RED_ON_POOL else nc.vector
            red_eng.tensor_reduce(o_v, t3, axis=X, op=ADD)

        # inv2 = 1/max(nv2, eps) in fp32; fp16 copy for the s-scale
        nc.vector.tensor_scalar_max(nv2[:], nv2[:], 1e-28)
        nc.vector.reciprocal(nv2[:], nv2[:])
        nc.vector.tensor_copy(inv2[:], nv2[:])

        # s6 = (d d^T) * inv2 - alpha * G
        nc.vector.tensor_mul(s6[:, :, 0:3], d3, d3)
        nc.vector.tensor_mul(s6[:, :, 3:6:2], d3[:, :, 0:2], d3[:, :, 1:3])
        nc.vector.tensor_mul(s6[:, :, 4], d3[:, :, 0], d3[:, :, 2])
        i2_b6 = inv2[:].unsqueeze(2).broadcast_to((P, ni, 6))
        nc.vector.tensor_mul(s6, s6, i2_b6)

        a_b6 = a_sb[:, i0:i1].unsqueeze(2).broadcast_to((P, ni, 6))
        ag_eng = nc.gpsimd if AG_ON_POOL else nc.vector
        ag_eng.tensor_mul(ag6, g6, a_b6)
        nc.vector.tensor_sub(s6, s6, ag6)

        s_h = s_aos[:].tensor
        s_w = s_aos[:].ap[0][0]  # per-partition alloc width (elements)
        for b in range(blk0, blk1):
            lo = 54 * b - 6 * i0  # local col offset in this chunk's s_aos
            pack96 = pkp.tile([P, 96], F16, tag="pk")
            pack_dst = pack96[:].rearrange("p (g c) -> p g c", c=32)
            pack_src = bass.AP(s_h, lo, [[s_w, P], [18, 3], [1, 32]])
            if PACK_ON_ACT:
                nc.scalar.copy(pack_dst, pack_src)
            else:
                nc.vector.tensor_copy(pack_dst, pack_src)
            st_ps = psq.tile([96, P], F16)
            nc.tensor.transpose(st_ps[:], pack96[:], id_sb[:])
            st_sb = stp.tile([96, P], F16)
            if ST_ON_ACT:
                nc.scalar.copy(st_sb[:], st_ps[:])
            else:
                nc.vector.tensor_copy(st_sb[:], st_ps[:])

            o_ps = psm.tile([P, 1536], F32, tag="mmout")
            for g in range(BLOCK_GROUPS):
                nc.tensor.matmul(
                    o_ps[:, 512 * g : 512 * g + 508],
                    st_sb[32 * g : 32 * g + 18, :],
                    q96_sb[32 * g : 32 * g + 18, :],
                    start=True,
                    stop=True,
                )
            stage = stagep.tile([P, 3 * 507], F16, tag="stage")
            o_h = o_ps[:].tensor
            o_off = o_ps[:].offset
            o_w = o_ps[:].ap[0][0]
            src = bass.AP(o_h, o_off, [[o_w, P], [512, 3], [1, 507]])
            dst = stage[:].rearrange("p (g k) -> p g k", k=507)
            if COPY_MOD and copy_idx % COPY_MOD == COPY_MOD - 1:
                nc.scalar.copy(dst, src)
            else:
                nc.vector.tensor_copy(dst, src)
            copy_idx += 1
            nc.sync.dma_start(
                out_flat[:, 1521 * b : 1521 * (b + 1)], stage[:]
            )

        if has_rem and REM_I:
            lo = 54 * N_BLOCKS - 6 * i0
            st_ps = psq.tile([96, P], F16)
            nc.tensor.transpose(
                st_ps[0 : 6 * REM_I, :], s_aos[:, lo : lo + 6 * REM_I], id_sb[:]
            )
            st_sb = stp.tile([96, P], F16)
            nc.vector.tensor_copy(st_sb[0 : 6 * REM_I, :], st_ps[0 : 6 * REM_I, :])
            o_ps = psm.tile([P, 1536], F32, tag="mmout")
            nc.tensor.matmul(
                o_ps[:, 0 : KG * REM_I],
                st_sb[0 : 6 * REM_I, :],
                q12_sb[:],
                start=True,
                stop=True,
            )
            stage = stagep.tile([P, 3 * 507], F16, tag="stage")
            nc.vector.tensor_copy(stage[:, 0 : KG * REM_I], o_ps[:, 0 : KG * REM_I])
            nc.sync.dma_start(
                out_flat[:, 1521 * N_BLOCKS :], stage[:, 0 : KG * REM_I]
            )


_NC_CACHE = {}


def _get_nc(reps=1):
    if reps not in _NC_CACHE:
        _NC_CACHE[reps] = build_nc(reps)
    return _NC_CACHE[reps]


def make_in_maps(eyes, v, R, t, alpha):
    q96 = make_q96()
    q12 = make_q12()
    ident = np.eye(P, dtype=np.float16)
    eyes = np.ascontiguousarray(eyes, np.float32).reshape(N_CORES, BC, 3)
    v = np.ascontiguousarray(v, np.float32).reshape(N_CORES, BC, 3)
    R = np.ascontiguousarray(R, np.float32).reshape(N_CORES, BC, 3, 3)
    t = np.ascontiguousarray(t, np.float32).reshape(N_CORES, BC, 3)
    alpha = np.ascontiguousarray(alpha, np.float32).reshape(N_CORES, BC)
    return [
        {
            "eyes": eyes[c], "v": v[c], "R": R[c], "t": t[c], "alpha": alpha[c],
            "q96": q96, "q12": q12, "ident": ident,
        }
        for c in range(N_CORES)
    ]


def kernel(eyes, v, R, t, alpha):
    nc = _get_nc(1)
    in_maps = make_in_maps(eyes, v, R, t, alpha)
    res = run_bass_kernel_spmd(nc, in_maps, list(range(N_CORES)))
    out = np.concatenate([res.results[c]["out"] for c in range(N_CORES)], axis=0)
    return out.astype(np.float32)



# revision 20
# speedup vs baseline: 1.1579x; 1.1579x over previous
"""Trainium2 Bass kernel for ConeProjection (v3).

Math (per batch element b):
    W     = [R[:,0], R[:,1], t - eyes]          (3 rows)
    d_a   = v . W_a          (unnormalized)
    G_ac  = W_a . W_c
    inv2  = 1 / ||v||^2
    s     = (d d^T) * inv2 - alpha * G          (6 unique entries)
    out[k] = s . q[k],  q[k] = [x^2, y^2, 1, 2xy, 2x, 2y]  (169 grid pts)

Strategy: pure data-parallel over 8 NeuronCores (batch 131072 -> 16384/core).
Per core, partition p holds batch [p*NI, (p+1)*NI); within-partition index i.
Inputs load via HWDGE as fp32 (one DMA per tensor; R split for ramp); the
first multiply level reads fp32 and writes fp16. Sigma entries are written
directly into a transpose-ready padded layout (24 used / 32 cols per group of
4 i's), so PE transposes [128,128] slices straight out of the elementwise
output. Each block = 16 i's -> 1 transpose + 4 row-tiled matmul groups
(K=24, N=2x338 fp32 PSUM); per-group PSUM->SBUF copies alternate DVE/ACT,
elementwise work is spread over DVE/ACT/Pool (latency-first on early chunks,
throughput-first later). One contiguous 692KB output DMA per block. Output
returns fp16, upcast to fp32 on the host.
"""

from contextlib import ExitStack, nullcontext

import numpy as np

import concourse.bass as bass
import concourse.bacc as bacc
import concourse.tile as tile
from concourse import mybir
from concourse.bass_utils import run_bass_kernel_spmd

N_CORES = 8
B = 131072
BC = B // N_CORES          # 16384 per core
P = 128                    # partitions
NI = BC // P               # 128 within-partition batch indices
KG = 169                   # grid points
F32 = mybir.dt.float32
F16 = mybir.dt.float16

GROUP = 4                  # i's per matmul group; K = 6*GROUP = 24
GPB = 4                    # groups per block (one [128,128] PE transpose)
IPB = GROUP * GPB          # 16 i per block
N_BLOCKS = NI // IPB       # 8
NMM = GROUP * KG           # 676 matmul free size
NMH = NMM // 2             # 338: matmul N split so fp32 out fits a PSUM bank
GW = 32                    # padded cols per group (24 used)

CFG = dict(
    CHUNKS=(1, 1, 1, 1, 2, 2),  # blocks per elementwise chunk
    R_SPLITS=(1, 1, 2, 4),  # R input loaded in these block-granular pieces
    RAMP_N=3,               # first chunks run latency-optimal (DVE heavy)
    STAGE_V=12,             # of the 32 staging copies, this many go to DVE
    SUB_ENG="p",            # s6 -= ag engine: p=pool, v=vector
    W2_ENG="p",             # w2 = t - eyes engine
    AG_ENG="p",             # ag = alpha*G engine
    MUL_ENG="p",            # cross-product muls engine (v=DVE, p=Pool)
    SQ_ENG="p",             # squares engine steady state (a=ACT, p=Pool)
    ADD1_ENG="p",           # dots first add engine
    ADD2_ENG="v",           # dots second add engine
    ST_ENG="v",             # S^T PSUM->SBUF copy engine (a=ACT, v=DVE)
    TAIL_SPLIT=True,        # split last block's output DMA in two
    INV2F32=True,           # feed fp32 inv2 straight into the e-mul
    TR_DMA=False,           # transpose via DMA xbar instead of PE
    IN_RING="a",            # input-load HWDGE ring: a=ACT (decoupled from
                            # the SP ring that carries output stores), s=SP
    PSM_BUFS=3,             # [P,1024] f32 tiles = 2 banks each; psq takes 2
    STAGE_BUFS=3,
    ST_BUFS=3,
)


def _grid_q():
    ii, jj = np.meshgrid(np.arange(13), np.arange(13), indexing="ij")
    x = ((ii - 6) / 6.0).reshape(-1)
    y = ((jj - 6) / 6.0).reshape(-1)
    q = np.stack([x * x, y * y, np.ones(KG), 2 * x * y, 2 * x, 2 * y], axis=0)
    return q.astype(np.float16)  # [6, 169]


def make_q128():
    """[128, 676]: K=24 block-diag Q replicated at partition bases 0/32/64/96."""
    q6 = _grid_q()
    q24 = np.zeros((6 * GROUP, NMM), np.float16)
    for a in range(GROUP):
        q24[6 * a : 6 * a + 6, KG * a : KG * a + KG] = q6
    out = np.zeros((P, NMM), np.float16)
    for g in range(GPB):
        out[32 * g : 32 * g + 6 * GROUP, :] = q24
    return out


def _eng(nc, code):
    return {"v": nc.vector, "a": nc.scalar, "p": nc.gpsimd}[code]


def build_nc(reps: int = 1, loop_n: int = 0, **cfg_over):
    cfg = dict(CFG)
    cfg.update(cfg_over)
    nc = bacc.Bacc("TRN2", target_bir_lowering=False, debug=False,
                   num_devices=N_CORES)

    eyes_d = nc.declare_dram_parameter("eyes", [BC, 3], F32, isOutput=False)
    v_d = nc.declare_dram_parameter("v", [BC, 3], F32, isOutput=False)
    r_d = nc.declare_dram_parameter("R", [BC, 3, 3], F32, isOutput=False)
    t_d = nc.declare_dram_parameter("t", [BC, 3], F32, isOutput=False)
    a_d = nc.declare_dram_parameter("alpha", [BC], F32, isOutput=False)
    q_d = nc.declare_dram_parameter("q128", [P, NMM], F16, isOutput=False)
    id_d = nc.declare_dram_parameter("ident", [P, P], F16, isOutput=False)
    out_d = nc.declare_dram_parameter("out", [BC, KG], F16, isOutput=True)

    with tile.TileContext(nc) as tc:
        with ExitStack() as ctx:
            const = ctx.enter_context(tc.tile_pool(name="const", bufs=1))
            q_sb = const.tile([P, NMM], F16)
            id_sb = const.tile([P, P], F16)

            def load_consts():
                nc.sync.dma_start(q_sb[:], q_d.ap())
                if not cfg["TR_DMA"]:
                    nc.sync.dma_start(id_sb[:], id_d.ap())

            pools = dict(
                io=ctx.enter_context(tc.tile_pool(name="io", bufs=2)),
                scr=ctx.enter_context(tc.tile_pool(name="scr", bufs=2)),
                spool=ctx.enter_context(tc.tile_pool(name="sp", bufs=2)),
                stp=ctx.enter_context(
                    tc.tile_pool(name="st", bufs=cfg["ST_BUFS"])),
                stagep=ctx.enter_context(
                    tc.tile_pool(name="stage", bufs=cfg["STAGE_BUFS"])),
            )
            if not cfg["TR_DMA"]:
                pools["psq"] = ctx.enter_context(
                    tc.tile_pool(name="psq", bufs=2, space="PSUM"))
            pools["psm"] = ctx.enter_context(
                tc.tile_pool(name="psm", bufs=cfg["PSM_BUFS"], space="PSUM"))

            args = (nc, tc, pools, eyes_d, v_d, r_d, t_d, a_d, out_d,
                    q_sb, id_sb, cfg)
            if loop_n:
                load_consts()
                with tc.For_i(0, loop_n, 1):
                    for _ in range(reps):
                        _emit_one_pass(*args)
            else:
                for rep in range(reps):
                    _emit_one_pass(*args, load_consts if rep == 0 else None)

    nc.compile()
    return nc


def _emit_one_pass(nc, tc, pools, eyes_d, v_d, r_d, t_d, a_d, out_d,
                   q_sb, id_sb, cfg, load_consts=None):
    with ExitStack() as lpctx:
        lpctx.enter_context(
            nc.allow_low_precision(reason="fp16 kernel validated vs fp32 ref"))
        _emit_one_pass_lp(nc, tc, pools, eyes_d, v_d, r_d, t_d, a_d, out_d,
                          q_sb, id_sb, cfg, load_consts)


def _emit_one_pass_lp(nc, tc, pools, eyes_d, v_d, r_d, t_d, a_d, out_d,
                      q_sb, id_sb, cfg, load_consts):
    X = mybir.AxisListType.X
    ADD = mybir.AluOpType.add

    io = pools["io"]
    scr = pools["scr"]
    spool = pools["spool"]
    stp = pools["stp"]
    stagep = pools["stagep"]
    psm = pools["psm"]

    # DRAM views (per-partition contiguous)
    eyes_f = eyes_d.ap().rearrange("(p i) c -> p (i c)", p=P)
    v_f = v_d.ap().rearrange("(p i) c -> p (i c)", p=P)
    r_f = r_d.ap().rearrange("(p i) a b -> p (i a b)", p=P)
    t_f = t_d.ap().rearrange("(p i) k -> p (i k)", p=P)
    out_flat = out_d.ap().rearrange("(p i) k -> p (i k)", p=P)  # [P, NI*KG]

    # alpha: SWDGE cast load straight to fp16 (Pool is idle at pass start)
    a16 = io.tile([P, NI], F16, tag="alpha")
    nc.gpsimd.dma_start(a16[:], a_d.ap().rearrange("(p i) -> p i", p=P))

    # chunk table: (i0, ni, blk0, nb)
    assert sum(cfg["CHUNKS"]) == N_BLOCKS
    chunks = []
    b0 = 0
    for nb in cfg["CHUNKS"]:
        chunks.append((IPB * b0, IPB * nb, b0, nb))
        b0 += nb

    # input DMAs (HWDGE, fp32): first-chunk tensors first, consts after,
    # then the R remainder — minimizes time-to-first-compute
    eyes32 = io.tile([P, 3 * NI], F32, tag="eyes")
    v32 = io.tile([P, 3 * NI], F32, tag="v")
    t32 = io.tile([P, 3 * NI], F32, tag="t")
    r32 = io.tile([P, 9 * NI], F32, tag="r")
    assert sum(cfg["R_SPLITS"]) == N_BLOCKS
    in_eng = nc.scalar if cfg["IN_RING"] == "a" else nc.sync
    in_eng.dma_start(v32[:], v_f[:])
    lo, hi = 0, 9 * IPB * cfg["R_SPLITS"][0]
    in_eng.dma_start(r32[:, lo:hi], r_f[:, lo:hi])
    in_eng.dma_start(eyes32[:], eyes_f[:])
    in_eng.dma_start(t32[:], t_f[:])
    if load_consts is not None:
        load_consts()
    rb0 = cfg["R_SPLITS"][0]
    for rs in cfg["R_SPLITS"][1:]:
        lo, hi = 9 * IPB * rb0, 9 * IPB * (rb0 + rs)
        in_eng.dma_start(r32[:, lo:hi], r_f[:, lo:hi])
        rb0 += rs

    mm_idx = 0
    for ci, (i0, ni, blk0, nb) in enumerate(chunks):
        ngr = ni // GROUP

        w2 = scr.tile([P, 3 * ni], F32, tag=f"w2_{ci}")
        prod = scr.tile([P, 30 * ni], F16, tag=f"prod{ci}")
        dots9 = scr.tile([P, 9 * ni], F16, tag=f"d9_{ci}")
        nv2 = scr.tile([P, ni], F32, tag=f"nv2{ci}")
        inv2 = scr.tile([P, ni], F16, tag=f"inv2{ci}")
        e3 = scr.tile([P, 3 * ni], F16, tag=f"e3_{ci}")
        ag = scr.tile([P, 6 * ni], F16, tag=f"ag_{ci}")
        s_pad = spool.tile([P, GW * ngr], F16, tag=f"sp{ci}")

        v3 = v32[:, 3 * i0 : 3 * (i0 + ni)].rearrange("p (i c) -> p i c", c=3)
        t3 = t32[:, 3 * i0 : 3 * (i0 + ni)]
        ey3 = eyes32[:, 3 * i0 : 3 * (i0 + ni)]
        w23 = w2[:].rearrange("p (i c) -> p i c", c=3)
        rb = r32[:, 9 * i0 : 9 * (i0 + ni)].rearrange(
            "p (i a b) -> p b i a", a=3, b=3)
        pr = prod[:].rearrange("p (s i c) -> p s i c", s=10, c=3)

        ramp = ci < cfg["RAMP_N"]
        mul_e = "v" if ramp else cfg["MUL_ENG"]
        sq_e = "a" if ramp else cfg["SQ_ENG"]
        add1_e = "v" if ramp else cfg["ADD1_ENG"]
        add2_e = "v" if ramp else cfg["ADD2_ENG"]
        ag_e = "v" if ramp else cfg["AG_ENG"]
        sub_e = "v" if ramp else cfg["SUB_ENG"]
        w2_e = "v" if ramp else cfg["W2_ENG"]

        # w2 = t - eyes (fp32)
        _eng(nc, w2_e).tensor_sub(w2[:], t3, ey3)

        # products (fp32 in -> fp16 out)
        v_b2 = v3.unsqueeze(1).broadcast_to((P, 2, ni, 3))
        w_b2 = w23.unsqueeze(1).broadcast_to((P, 2, ni, 3))
        me = _eng(nc, mul_e)
        me.tensor_mul(pr[:, 1:3], v_b2, rb[:, 0:2])             # v.r0, v.r1
        me.tensor_mul(pr[:, 3], v3, w23)                        # v.w2
        me.tensor_mul(pr[:, 7], rb[:, 0], rb[:, 1])             # r0.r1
        me.tensor_mul(pr[:, 8:10], rb[:, 0:2], w_b2)            # r0.w2, r1.w2
        if sq_e == "a":
            nc.scalar.square(pr[:, 0], v3)                      # v.v
            nc.scalar.square(pr[:, 4:6], rb[:, 0:2])            # r0.r0, r1.r1
            nc.scalar.square(pr[:, 6], w23)                     # w2.w2
        else:
            se = _eng(nc, sq_e)
            se.tensor_mul(pr[:, 0], v3, v3)
            se.tensor_mul(pr[:, 4:6], rb[:, 0:2], rb[:, 0:2])
            se.tensor_mul(pr[:, 6], w23, w23)

        # dots: nv2 via reduce (fp32); the 9 dots via two strided adds
        # (d-major [9, ni] layout)
        d9 = dots9[:].rearrange("p (s i) -> p s i", s=9)
        nc.vector.tensor_reduce(nv2[:], pr[:, 0], axis=X, op=ADD)
        _eng(nc, add1_e).tensor_add(d9, pr[:, 1:10, :, 0], pr[:, 1:10, :, 1])
        _eng(nc, add2_e).tensor_add(d9, d9, pr[:, 1:10, :, 2])

        # inv2 = 1/nv2 (fp32; ||v||^2 >= 0.079 for these inputs)
        nc.vector.reciprocal(nv2[:], nv2[:])
        if not cfg["INV2F32"]:
            nc.vector.tensor_copy(inv2[:], nv2[:])

        # e = d * inv2 (fp16, [3, ni])
        e3v = e3[:].rearrange("p (s i) -> p s i", s=3)
        i_src = nv2 if cfg["INV2F32"] else inv2
        i_b3 = i_src[:].unsqueeze(1).broadcast_to((P, 3, ni))
        nc.vector.tensor_mul(e3v, d9[:, 0:3], i_b3)

        # raw-AP helpers over the padded sigma layout:
        # col(i=GROUP*gg+j, c) = GW*gg + 6*j + c
        s_h = s_pad[:].tensor
        s_o = s_pad[:].offset
        s_w = s_pad[:].ap[0][0]

        def sp_ap(c0, cn):
            return bass.AP(s_h, s_o + c0,
                           [[s_w, P], [GW, ngr], [6, GROUP], [1, cn]])

        d_h = dots9[:].tensor
        d_o = dots9[:].offset
        d_w = dots9[:].ap[0][0]

        def d_ap(s0, sn, s_stride=None):
            st = ni if s_stride is None else s_stride
            return bass.AP(d_h, d_o + s0 * ni,
                           [[d_w, P], [GROUP, ngr], [1, GROUP], [st, sn]])

        e_h = e3[:].tensor
        e_o = e3[:].offset
        e_w = e3[:].ap[0][0]

        def e_ap(s0, sn, s_stride=None):
            st = ni if s_stride is None else s_stride
            return bass.AP(e_h, e_o + s0 * ni,
                           [[e_w, P], [GROUP, ngr], [1, GROUP], [st, sn]])

        # s6 entries: diag c=0..2: e_c*d_c; c=3: e0*d1, c=4: e0*d2, c=5: e1*d2
        nc.vector.tensor_mul(sp_ap(0, 3), e_ap(0, 3), d_ap(0, 3))
        nc.vector.tensor_mul(sp_ap(3, 2), e_ap(0, 2, 0), d_ap(1, 2))
        nc.vector.tensor_mul(sp_ap(5, 1), e_ap(1, 1), d_ap(2, 1))

        # ag = alpha * G  (fp16, [6, ni] c-major)
        ag6 = ag[:].rearrange("p (s i) -> p s i", s=6)
        a_b6 = a16[:, i0 : i0 + ni].unsqueeze(1).broadcast_to((P, 6, ni))
        _eng(nc, ag_e).tensor_mul(ag6, d9[:, 3:9], a_b6)

        # s6 -= ag
        ag_h = ag[:].tensor
        ag_o = ag[:].offset
        ag_w = ag[:].ap[0][0]
        ag_p = bass.AP(ag_h, ag_o,
                       [[ag_w, P], [GROUP, ngr], [1, GROUP], [ni, 6]])
        _eng(nc, sub_e).tensor_sub(sp_ap(0, 6), sp_ap(0, 6), ag_p)

        # blocks: S^T transpose + 4 row-tiled matmuls + staged copies + DMA
        for lb in range(nb):
            b = blk0 + lb
            st_sb = stp.tile([P, P], F16, tag="stsb")
            s_slice = s_pad[:, P * lb : P * (lb + 1)]
            if cfg["TR_DMA"]:
                nc.scalar.dma_start_transpose(st_sb[:], s_slice)
            else:
                st_ps = pools["psq"].tile([P, P], F16, tag="stps")
                nc.tensor.transpose(st_ps[:], s_slice, id_sb[:])
                if cfg["ST_ENG"] == "a":
                    nc.scalar.copy(st_sb[:], st_ps[:])
                else:
                    nc.vector.tensor_copy(st_sb[:], st_ps[:])

            stage = stagep.tile([P, IPB * KG], F16, tag="stage")
            for g in range(GPB):
                o_ps = psm.tile([P, 1024], F32, tag="mmout")
                for h in range(2):
                    nc.tensor.matmul(
                        o_ps[:, 512 * h : 512 * h + NMH],
                        st_sb[32 * g : 32 * g + 6 * GROUP, :],
                        q_sb[32 * g : 32 * g + 6 * GROUP,
                             NMH * h : NMH * (h + 1)],
                        start=True,
                        stop=True,
                        tile_position=(32 * g, 0),
                    )
                if b == N_BLOCKS - 1:
                    eng = "v" if g % 2 == 0 else "a"  # tail: both engines
                else:
                    eng = ("v" if (mm_idx * cfg["STAGE_V"]) % 32
                           < cfg["STAGE_V"] else "a")
                mm_idx += 1
                o_h = o_ps[:].tensor
                o_o = o_ps[:].offset
                o_w = o_ps[:].ap[0][0]
                src = bass.AP(o_h, o_o, [[o_w, P], [512, 2], [1, NMH]])
                dst = stage[:, NMM * g : NMM * (g + 1)].rearrange(
                    "p (h k) -> p h k", h=2)
                if eng == "a":
                    nc.scalar.copy(dst, src)
                else:
                    nc.vector.tensor_copy(dst, src)
                if cfg["TAIL_SPLIT"] and b == N_BLOCKS - 1 and g == 1:
                    nc.sync.dma_start(
                        out_flat[:, IPB * KG * b : IPB * KG * b + 2 * NMM],
                        stage[:, 0 : 2 * NMM],
                    )
            if cfg["TAIL_SPLIT"] and b == N_BLOCKS - 1:
                nc.sync.dma_start(
                    out_flat[:, IPB * KG * b + 2 * NMM : IPB * KG * (b + 1)],
                    stage[:, 2 * NMM :],
                )
            else:
                nc.sync.dma_start(
                    out_flat[:, IPB * KG * b : IPB * KG * (b + 1)], stage[:]
                )


_NC_CACHE = {}


def _get_nc(reps=1):
    if reps not in _NC_CACHE:
        _NC_CACHE[reps] = build_nc(reps)
    return _NC_CACHE[reps]


def make_in_maps(eyes, v, R, t, alpha):
    q128 = make_q128()
    ident = np.eye(P, dtype=np.float16)
    eyes = np.ascontiguousarray(eyes, np.float32).reshape(N_CORES, BC, 3)
    v = np.ascontiguousarray(v, np.float32).reshape(N_CORES, BC, 3)
    R = np.ascontiguousarray(R, np.float32).reshape(N_CORES, BC, 3, 3)
    t = np.ascontiguousarray(t, np.float32).reshape(N_CORES, BC, 3)
    alpha = np.ascontiguousarray(alpha, np.float32).reshape(N_CORES, BC)
    return [
        {
            "eyes": eyes[c], "v": v[c], "R": R[c], "t": t[c], "alpha": alpha[c],
            "q128": q128, "ident": ident,
        }
        for c in range(N_CORES)
    ]


def kernel(eyes, v, R, t, alpha):
    nc = _get_nc(1)
    in_maps = make_in_maps(eyes, v, R, t, alpha)
    res = run_bass_kernel_spmd(nc, in_maps, list(range(N_CORES)))
    out = np.concatenate([res.results[c]["out"] for c in range(N_CORES)], axis=0)
    return out.astype(np.float32)


# revision 25
# speedup vs baseline: 1.2464x; 1.0764x over previous
"""Trainium2 Bass kernel for ConeProjection (v3).

Math (per batch element b):
    W     = [R[:,0], R[:,1], t - eyes]          (3 rows)
    d_a   = v . W_a          (unnormalized)
    G_ac  = W_a . W_c
    inv2  = 1 / ||v||^2
    s     = (d d^T) * inv2 - alpha * G          (6 unique entries)
    out[k] = s . q[k],  q[k] = [x^2, y^2, 1, 2xy, 2x, 2y]  (169 grid pts)

Strategy: pure data-parallel over 8 NeuronCores (batch 131072 -> 16384/core).
Per core, partition p holds batch [p*NI, (p+1)*NI); within-partition index i.
Inputs load via HWDGE as fp32 (one DMA per tensor; R split for ramp); the
first multiply level reads fp32 and writes fp16. Sigma entries are written
directly into a transpose-ready padded layout (24 used / 32 cols per group of
4 i's), so PE transposes [128,128] slices straight out of the elementwise
output. Each block = 16 i's -> 1 transpose + 4 row-tiled matmul groups
(K=24, N=2x338 fp32 PSUM); per-group PSUM->SBUF copies alternate DVE/ACT,
elementwise work is spread over DVE/ACT/Pool (latency-first on early chunks,
throughput-first later). One contiguous 692KB output DMA per block. Output
returns fp16, upcast to fp32 on the host.
"""

from contextlib import ExitStack, nullcontext

import numpy as np

import concourse.bass as bass
import concourse.bacc as bacc
import concourse.tile as tile
from concourse import mybir
from concourse.bass_utils import run_bass_kernel_spmd

N_CORES = 8
B = 131072
BC = B // N_CORES          # 16384 per core
P = 128                    # partitions
NI = BC // P               # 128 within-partition batch indices
KG = 169                   # grid points
F32 = mybir.dt.float32
F16 = mybir.dt.float16

GROUP = 4                  # i's per matmul group; K = 6*GROUP = 24
GPB = 4                    # groups per block (one [128,128] PE transpose)
IPB = GROUP * GPB          # 16 i per block
N_BLOCKS = NI // IPB       # 8
NMM = GROUP * KG           # 676 matmul free size
NMH = NMM // 2             # 338: matmul N split so fp32 out fits a PSUM bank
GW = 32                    # padded cols per group (24 used)

CFG = dict(
    CHUNKS=(1, 1, 1, 1, 2, 2),  # blocks per elementwise chunk
    R_SPLITS=(1, 1, 2, 4),  # R input loaded in these block-granular pieces
    RAMP_N=3,               # first chunks run latency-optimal (DVE heavy)
    STAGE_V=12,             # of the 32 staging copies, this many go to DVE
    SUB_ENG="p",            # s6 -= ag engine: p=pool, v=vector
    W2_ENG="p",             # w2 = t - eyes engine
    AG_ENG="p",             # ag = alpha*G engine
    MUL_ENG="p",            # cross-product muls engine (v=DVE, p=Pool)
    SQ_ENG="p",             # squares engine steady state (a=ACT, p=Pool)
    ADD1_ENG="p",           # dots first add engine
    ADD2_ENG="v",           # dots second add engine
    ST_ENG="v",             # S^T PSUM->SBUF copy engine (a=ACT, v=DVE)
    TAIL_SPLIT=True,        # split last block's output DMA in two
    INV2F32=True,           # feed fp32 inv2 straight into the e-mul
    TR_DMA=False,           # transpose via DMA xbar instead of PE
    STAGGER=True,           # staggered sem reset in For_i timing loops
                            # (avoids the per-iteration all-engine barrier)
    IN_RING="s",            # input-load HWDGE ring (a=ACT, s=SP)
    RAMP_SQ="a",            # squares engine during ramp chunks
    OUT_RING="s",           # output-store HWDGE ring (a=ACT, s=SP); keeping
                            # the rings separate lets pass N+1 inputs dispatch
                            # while pass N outputs drain
    PSM_BUFS=3,             # [P,1024] f32 tiles = 2 banks each; psq takes 2
    STAGE_BUFS=3,
    ST_BUFS=3,
)


def _grid_q():
    ii, jj = np.meshgrid(np.arange(13), np.arange(13), indexing="ij")
    x = ((ii - 6) / 6.0).reshape(-1)
    y = ((jj - 6) / 6.0).reshape(-1)
    q = np.stack([x * x, y * y, np.ones(KG), 2 * x * y, 2 * x, 2 * y], axis=0)
    return q.astype(np.float16)  # [6, 169]


def make_q128():
    """[128, 676]: K=24 block-diag Q replicated at partition bases 0/32/64/96."""
    q6 = _grid_q()
    q24 = np.zeros((6 * GROUP, NMM), np.float16)
    for a in range(GROUP):
        q24[6 * a : 6 * a + 6, KG * a : KG * a + KG] = q6
    out = np.zeros((P, NMM), np.float16)
    for g in range(GPB):
        out[32 * g : 32 * g + 6 * GROUP, :] = q24
    return out


def _eng(nc, code):
    return {"v": nc.vector, "a": nc.scalar, "p": nc.gpsimd}[code]


def build_nc(reps: int = 1, loop_n: int = 0, **cfg_over):
    cfg = dict(CFG)
    cfg.update(cfg_over)
    nc = bacc.Bacc("TRN2", target_bir_lowering=False, debug=False,
                   num_devices=N_CORES)

    eyes_d = nc.declare_dram_parameter("eyes", [BC, 3], F32, isOutput=False)
    v_d = nc.declare_dram_parameter("v", [BC, 3], F32, isOutput=False)
    r_d = nc.declare_dram_parameter("R", [BC, 3, 3], F32, isOutput=False)
    t_d = nc.declare_dram_parameter("t", [BC, 3], F32, isOutput=False)
    a_d = nc.declare_dram_parameter("alpha", [BC], F32, isOutput=False)
    q_d = nc.declare_dram_parameter("q128", [P, NMM], F16, isOutput=False)
    id_d = nc.declare_dram_parameter("ident", [P, P], F16, isOutput=False)
    out_d = nc.declare_dram_parameter("out", [BC, KG], F16, isOutput=True)

    with tile.TileContext(nc) as tc:
        with ExitStack() as ctx:
            const = ctx.enter_context(tc.tile_pool(name="const", bufs=1))
            q_sb = const.tile([P, NMM], F16)
            id_sb = const.tile([P, P], F16)

            def load_consts():
                nc.sync.dma_start(q_sb[:], q_d.ap())
                if not cfg["TR_DMA"]:
                    nc.sync.dma_start(id_sb[:], id_d.ap())

            pools = dict(
                io=ctx.enter_context(tc.tile_pool(name="io", bufs=2)),
                scr=ctx.enter_context(tc.tile_pool(name="scr", bufs=2)),
                spool=ctx.enter_context(tc.tile_pool(name="sp", bufs=2)),
                stp=ctx.enter_context(
                    tc.tile_pool(name="st", bufs=cfg["ST_BUFS"])),
                stagep=ctx.enter_context(
                    tc.tile_pool(name="stage", bufs=cfg["STAGE_BUFS"])),
            )
            if not cfg["TR_DMA"]:
                pools["psq"] = ctx.enter_context(
                    tc.tile_pool(name="psq", bufs=2, space="PSUM"))
            pools["psm"] = ctx.enter_context(
                tc.tile_pool(name="psm", bufs=cfg["PSM_BUFS"], space="PSUM"))

            args = (nc, tc, pools, eyes_d, v_d, r_d, t_d, a_d, out_d,
                    q_sb, id_sb, cfg)
            if loop_n:
                load_consts()
                with tc.For_i(0, loop_n, 1,
                              staggered_reset=cfg["STAGGER"]):
                    for _ in range(reps):
                        _emit_one_pass(*args)
            else:
                for rep in range(reps):
                    _emit_one_pass(*args, load_consts if rep == 0 else None)

    nc.compile()
    return nc


def _emit_one_pass(nc, tc, pools, eyes_d, v_d, r_d, t_d, a_d, out_d,
                   q_sb, id_sb, cfg, load_consts=None):
    with ExitStack() as lpctx:
        lpctx.enter_context(
            nc.allow_low_precision(reason="fp16 kernel validated vs fp32 ref"))
        _emit_one_pass_lp(nc, tc, pools, eyes_d, v_d, r_d, t_d, a_d, out_d,
                          q_sb, id_sb, cfg, load_consts)


def _emit_one_pass_lp(nc, tc, pools, eyes_d, v_d, r_d, t_d, a_d, out_d,
                      q_sb, id_sb, cfg, load_consts):
    X = mybir.AxisListType.X
    ADD = mybir.AluOpType.add

    out_eng = nc.scalar if cfg["OUT_RING"] == "a" else nc.sync
    io = pools["io"]
    scr = pools["scr"]
    spool = pools["spool"]
    stp = pools["stp"]
    stagep = pools["stagep"]
    psm = pools["psm"]

    # DRAM views (per-partition contiguous)
    eyes_f = eyes_d.ap().rearrange("(p i) c -> p (i c)", p=P)
    v_f = v_d.ap().rearrange("(p i) c -> p (i c)", p=P)
    r_f = r_d.ap().rearrange("(p i) a b -> p (i a b)", p=P)
    t_f = t_d.ap().rearrange("(p i) k -> p (i k)", p=P)
    out_flat = out_d.ap().rearrange("(p i) k -> p (i k)", p=P)  # [P, NI*KG]

    # alpha: SWDGE cast load straight to fp16 (Pool is idle at pass start)
    a16 = io.tile([P, NI], F16, tag="alpha")
    nc.gpsimd.dma_start(a16[:], a_d.ap().rearrange("(p i) -> p i", p=P))

    # chunk table: (i0, ni, blk0, nb)
    assert sum(cfg["CHUNKS"]) == N_BLOCKS
    chunks = []
    b0 = 0
    for nb in cfg["CHUNKS"]:
        chunks.append((IPB * b0, IPB * nb, b0, nb))
        b0 += nb

    # input DMAs (HWDGE, fp32): first-chunk tensors first, consts after,
    # then the R remainder — minimizes time-to-first-compute
    eyes32 = io.tile([P, 3 * NI], F32, tag="eyes")
    v32 = io.tile([P, 3 * NI], F32, tag="v")
    t32 = io.tile([P, 3 * NI], F32, tag="t")
    r32 = io.tile([P, 9 * NI], F32, tag="r")
    assert sum(cfg["R_SPLITS"]) == N_BLOCKS
    in_eng = nc.scalar if cfg["IN_RING"] == "a" else nc.sync
    in_eng.dma_start(v32[:], v_f[:])
    lo, hi = 0, 9 * IPB * cfg["R_SPLITS"][0]
    in_eng.dma_start(r32[:, lo:hi], r_f[:, lo:hi])
    in_eng.dma_start(eyes32[:], eyes_f[:])
    in_eng.dma_start(t32[:], t_f[:])
    if load_consts is not None:
        load_consts()
    rb0 = cfg["R_SPLITS"][0]
    for rs in cfg["R_SPLITS"][1:]:
        lo, hi = 9 * IPB * rb0, 9 * IPB * (rb0 + rs)
        in_eng.dma_start(r32[:, lo:hi], r_f[:, lo:hi])
        rb0 += rs

    mm_idx = 0
    for ci, (i0, ni, blk0, nb) in enumerate(chunks):
        ngr = ni // GROUP

        w2 = scr.tile([P, 3 * ni], F32, tag=f"w2_{ci}")
        prod = scr.tile([P, 30 * ni], F16, tag=f"prod{ci}")
        dots9 = scr.tile([P, 9 * ni], F16, tag=f"d9_{ci}")
        nv2 = scr.tile([P, ni], F32, tag=f"nv2{ci}")
        inv2 = scr.tile([P, ni], F16, tag=f"inv2{ci}")
        e3 = scr.tile([P, 3 * ni], F16, tag=f"e3_{ci}")
        ag = scr.tile([P, 6 * ni], F16, tag=f"ag_{ci}")
        s_pad = spool.tile([P, GW * ngr], F16, tag=f"sp{ci}")

        v3 = v32[:, 3 * i0 : 3 * (i0 + ni)].rearrange("p (i c) -> p i c", c=3)
        t3 = t32[:, 3 * i0 : 3 * (i0 + ni)]
        ey3 = eyes32[:, 3 * i0 : 3 * (i0 + ni)]
        w23 = w2[:].rearrange("p (i c) -> p i c", c=3)
        rb = r32[:, 9 * i0 : 9 * (i0 + ni)].rearrange(
            "p (i a b) -> p b i a", a=3, b=3)
        pr = prod[:].rearrange("p (s i c) -> p s i c", s=10, c=3)

        ramp = ci < cfg["RAMP_N"]
        mul_e = "v" if ramp else cfg["MUL_ENG"]
        sq_e = cfg["RAMP_SQ"] if ramp else cfg["SQ_ENG"]
        add1_e = "v" if ramp else cfg["ADD1_ENG"]
        add2_e = "v" if ramp else cfg["ADD2_ENG"]
        ag_e = "v" if ramp else cfg["AG_ENG"]
        sub_e = "v" if ramp else cfg["SUB_ENG"]
        w2_e = "v" if ramp else cfg["W2_ENG"]

        # products (fp32 in -> fp16 out); v/R-only ones first so they can
        # start before eyes/t (and hence w2) are resident
        v_b2 = v3.unsqueeze(1).broadcast_to((P, 2, ni, 3))
        w_b2 = w23.unsqueeze(1).broadcast_to((P, 2, ni, 3))
        me = _eng(nc, mul_e)
        me.tensor_mul(pr[:, 1:3], v_b2, rb[:, 0:2])             # v.r0, v.r1
        me.tensor_mul(pr[:, 7], rb[:, 0], rb[:, 1])             # r0.r1
        if sq_e == "a":
            nc.scalar.square(pr[:, 0], v3)                      # v.v
            nc.scalar.square(pr[:, 4:6], rb[:, 0:2])            # r0.r0, r1.r1
        else:
            se = _eng(nc, sq_e)
            se.tensor_mul(pr[:, 0], v3, v3)
            se.tensor_mul(pr[:, 4:6], rb[:, 0:2], rb[:, 0:2])

        # w2 = t - eyes (fp32), then the w2-dependent products
        _eng(nc, w2_e).tensor_sub(w2[:], t3, ey3)
        me.tensor_mul(pr[:, 3], v3, w23)                        # v.w2
        me.tensor_mul(pr[:, 8:10], rb[:, 0:2], w_b2)            # r0.w2, r1.w2
        if sq_e == "a":
            nc.scalar.square(pr[:, 6], w23)                     # w2.w2
        else:
            se.tensor_mul(pr[:, 6], w23, w23)

        # dots: nv2 via reduce (fp32); the 9 dots via two strided adds
        # (d-major [9, ni] layout)
        d9 = dots9[:].rearrange("p (s i) -> p s i", s=9)
        nc.vector.tensor_reduce(nv2[:], pr[:, 0], axis=X, op=ADD)
        _eng(nc, add1_e).tensor_add(d9, pr[:, 1:10, :, 0], pr[:, 1:10, :, 1])
        _eng(nc, add2_e).tensor_add(d9, d9, pr[:, 1:10, :, 2])

        # inv2 = 1/nv2 (fp32; ||v||^2 >= 0.079 for these inputs)
        nc.vector.reciprocal(nv2[:], nv2[:])
        if not cfg["INV2F32"]:
            nc.vector.tensor_copy(inv2[:], nv2[:])

        # e = d * inv2 (fp16, [3, ni])
        e3v = e3[:].rearrange("p (s i) -> p s i", s=3)
        i_src = nv2 if cfg["INV2F32"] else inv2
        i_b3 = i_src[:].unsqueeze(1).broadcast_to((P, 3, ni))
        nc.vector.tensor_mul(e3v, d9[:, 0:3], i_b3)

        # raw-AP helpers over the padded sigma layout:
        # col(i=GROUP*gg+j, c) = GW*gg + 6*j + c
        s_h = s_pad[:].tensor
        s_o = s_pad[:].offset
        s_w = s_pad[:].ap[0][0]

        def sp_ap(c0, cn):
            return bass.AP(s_h, s_o + c0,
                           [[s_w, P], [GW, ngr], [6, GROUP], [1, cn]])

        d_h = dots9[:].tensor
        d_o = dots9[:].offset
        d_w = dots9[:].ap[0][0]

        def d_ap(s0, sn, s_stride=None):
            st = ni if s_stride is None else s_stride
            return bass.AP(d_h, d_o + s0 * ni,
                           [[d_w, P], [GROUP, ngr], [1, GROUP], [st, sn]])

        e_h = e3[:].tensor
        e_o = e3[:].offset
        e_w = e3[:].ap[0][0]

        def e_ap(s0, sn, s_stride=None):
            st = ni if s_stride is None else s_stride
            return bass.AP(e_h, e_o + s0 * ni,
                           [[e_w, P], [GROUP, ngr], [1, GROUP], [st, sn]])

        # s6 entries: diag c=0..2: e_c*d_c; c=3: e0*d1, c=4: e0*d2, c=5: e1*d2
        nc.vector.tensor_mul(sp_ap(0, 3), e_ap(0, 3), d_ap(0, 3))
        nc.vector.tensor_mul(sp_ap(3, 2), e_ap(0, 2, 0), d_ap(1, 2))
        nc.vector.tensor_mul(sp_ap(5, 1), e_ap(1, 1), d_ap(2, 1))

        # ag = alpha * G  (fp16, [6, ni] c-major)
        ag6 = ag[:].rearrange("p (s i) -> p s i", s=6)
        a_b6 = a16[:, i0 : i0 + ni].unsqueeze(1).broadcast_to((P, 6, ni))
        _eng(nc, ag_e).tensor_mul(ag6, d9[:, 3:9], a_b6)

        # s6 -= ag
        ag_h = ag[:].tensor
        ag_o = ag[:].offset
        ag_w = ag[:].ap[0][0]
        ag_p = bass.AP(ag_h, ag_o,
                       [[ag_w, P], [GROUP, ngr], [1, GROUP], [ni, 6]])
        _eng(nc, sub_e).tensor_sub(sp_ap(0, 6), sp_ap(0, 6), ag_p)

        # blocks: S^T transpose + 4 row-tiled matmuls + staged copies + DMA
        for lb in range(nb):
            b = blk0 + lb
            st_sb = stp.tile([P, P], F16, tag="stsb")
            s_slice = s_pad[:, P * lb : P * (lb + 1)]
            if cfg["TR_DMA"]:
                nc.scalar.dma_start_transpose(st_sb[:], s_slice)
            else:
                st_ps = pools["psq"].tile([P, P], F16, tag="stps")
                nc.tensor.transpose(st_ps[:], s_slice, id_sb[:])
                if cfg["ST_ENG"] == "a":
                    nc.scalar.copy(st_sb[:], st_ps[:])
                else:
                    nc.vector.tensor_copy(st_sb[:], st_ps[:])

            stage = stagep.tile([P, IPB * KG], F16, tag="stage")
            for g in range(GPB):
                o_ps = psm.tile([P, 1024], F32, tag="mmout")
                for h in range(2):
                    nc.tensor.matmul(
                        o_ps[:, 512 * h : 512 * h + NMH],
                        st_sb[32 * g : 32 * g + 6 * GROUP, :],
                        q_sb[32 * g : 32 * g + 6 * GROUP,
                             NMH * h : NMH * (h + 1)],
                        start=True,
                        stop=True,
                        tile_position=(32 * g, 0),
                    )
                if b == N_BLOCKS - 1:
                    eng = "v" if g % 2 == 0 else "a"  # tail: both engines
                else:
                    eng = ("v" if (mm_idx * cfg["STAGE_V"]) % 32
                           < cfg["STAGE_V"] else "a")
                mm_idx += 1
                o_h = o_ps[:].tensor
                o_o = o_ps[:].offset
                o_w = o_ps[:].ap[0][0]
                src = bass.AP(o_h, o_o, [[o_w, P], [512, 2], [1, NMH]])
                dst = stage[:, NMM * g : NMM * (g + 1)].rearrange(
                    "p (h k) -> p h k", h=2)
                if eng == "a":
                    nc.scalar.copy(dst, src)
                else:
                    nc.vector.tensor_copy(dst, src)
                if cfg["TAIL_SPLIT"] and b == N_BLOCKS - 1 and g == 1:
                    out_eng.dma_start(
                        out_flat[:, IPB * KG * b : IPB * KG * b + 2 * NMM],
                        stage[:, 0 : 2 * NMM],
                    )
            if cfg["TAIL_SPLIT"] and b == N_BLOCKS - 1:
                out_eng.dma_start(
                    out_flat[:, IPB * KG * b + 2 * NMM : IPB * KG * (b + 1)],
                    stage[:, 2 * NMM :],
                )
            else:
                out_eng.dma_start(
                    out_flat[:, IPB * KG * b : IPB * KG * (b + 1)], stage[:]
                )


_NC_CACHE = {}


def _get_nc(reps=1):
    if reps not in _NC_CACHE:
        _NC_CACHE[reps] = build_nc(reps)
    return _NC_CACHE[reps]


def make_in_maps(eyes, v, R, t, alpha):
    q128 = make_q128()
    ident = np.eye(P, dtype=np.float16)
    eyes = np.ascontiguousarray(eyes, np.float32).reshape(N_CORES, BC, 3)
    v = np.ascontiguousarray(v, np.float32).reshape(N_CORES, BC, 3)
    R = np.ascontiguousarray(R, np.float32).reshape(N_CORES, BC, 3, 3)
    t = np.ascontiguousarray(t, np.float32).reshape(N_CORES, BC, 3)
    alpha = np.ascontiguousarray(alpha, np.float32).reshape(N_CORES, BC)
    return [
        {
            "eyes": eyes[c], "v": v[c], "R": R[c], "t": t[c], "alpha": alpha[c],
            "q128": q128, "ident": ident,
        }
        for c in range(N_CORES)
    ]


def kernel(eyes, v, R, t, alpha):
    nc = _get_nc(1)
    in_maps = make_in_maps(eyes, v, R, t, alpha)
    res = run_bass_kernel_spmd(nc, in_maps, list(range(N_CORES)))
    out = np.concatenate([res.results[c]["out"] for c in range(N_CORES)], axis=0)
    return out.astype(np.float32)
